# revision 2
# baseline (speedup 1.0000x reference)
"""Trainium2 Bass kernel for LocalCrossCorrelationWithSmoothnessLoss.

Full inputs in, full output out. Internally: pure data-parallel over the
batch dim (B=8 -> 8 NeuronCores); each core computes partial sums for its
image; the host combines them into the three scalar losses.

Per-core pipeline (one 1024x1024 image pair + two flow channels):
  products   IJ = I*J (DVE), I2 = I^2, J2 = J^2 (ACT), bf16
  stage 1    H-direction 9-tap box conv as banded matmuls on the PE
             (band stationary, map moving, bf16, fp32 accumulate).
             Product maps use an 81-scaled band so the later combine is
             pure tensor-tensor work (81*S_IJ - S_I*S_J etc.).
  transpose  PE transpose per 120-row chunk (chunk-aligned, w-halo baked
             into the source free-dim offsets)
  stage 2    W-direction box conv, same banded matmuls on transposed maps
  combine    crossN = 81S_IJ - S_I*S_J, IvarN = 81S_II - S_I^2,
             JvarN = 81S_JJ - S_J^2   (all plain TT)
             cc = crossN^2 * exp(-ln(IvarN*JvarN))   (ln/exp on ACT, fp32)
             accumulated per-partition via STT accum_out
  smooth     sum(s^2) (ACT accum), lag products sum(s[w]s[w+1]) and
             sum(s[h]s[h+1]) (STT accum; row shift via SBUF->SBUF DMA).
             Edge-column/row corrections are computed on the host.

Output per core: 8 partial sums. Host assembles the losses in float64.
"""
import sys
import types
import numpy as np

sys.path.insert(0, "/opt/trn_rl_repo")

import ml_dtypes
import bass_rust
import concourse.bass as bass
import concourse.tile as tile
from concourse import mybir
from concourse import bass_utils
from concourse import tile_utils

F32 = mybir.dt.float32
F32R = mybir.dt.float32r
BF16 = mybir.dt.bfloat16
ALU = mybir.AluOpType
ACTF = mybir.ActivationFunctionType

H = 1024
W = 1024
PAD = 4
WIN = 81.0
ALPHA = 0.01
EPS = 1e-9
STRIDE = 120

# chunk table: (out_lo, out_n, in_lo, in_n)
CHUNKS = []
for _c in range((H + STRIDE - 1) // STRIDE):
    _olo = STRIDE * _c
    _on = min(STRIDE, H - _olo)
    _ilo = max(0, _olo - PAD)
    _ihi = min(H, _olo + _on + PAD)
    CHUNKS.append((_olo, _on, _ilo, _ihi - _ilo))
NCH = len(CHUNKS)

# allow using the full usable SBUF (tile_utils default is stale at 192K)
tile_utils.max_sbuf_usage = 206 * 1024

_nc_cache = {}


def _legalize_waits(nc, max_waits=1):
    """walrus here accepts only one sync-wait command per instruction;
    split extras onto same-engine NoOps placed just before."""
    ctr = 0
    for f in nc.m.functions:
        for bb in f.blocks:
            insts = bb.instructions
            i = 0
            while i < len(insts):
                ins = insts[i]
                si = ins.sync_info
                if si is None:
                    i += 1
                    continue
                w = list(si.on_wait)
                if len(w) <= max_waits:
                    i += 1
                    continue
                extra, keep = w[:-max_waits], w[-max_waits:]
                nops = []
                for j in range(0, len(extra), max_waits):
                    chunk = extra[j:j + max_waits]
                    nop = mybir.InstNoOp(name=f"I-wsplit-{ctr}", ins=[], outs=[])
                    ctr += 1
                    nop.engine = ins.engine
                    nop.sync_info = bass_rust.SyncInfo(on_wait=chunk, on_update=[])
                    nops.append(nop)
                ins.sync_info = bass_rust.SyncInfo(on_wait=keep,
                                                  on_update=list(si.on_update))
                insts[i:i] = nops
                i += len(nops) + 1


def _make_host_consts():
    """Band matrices (bf16), identity (bf16), ones (f32)."""
    def band(klo, kn, olo, on, scale):
        k = np.arange(klo, klo + kn)[:, None]
        m = np.arange(olo, olo + on)[None, :]
        return (np.abs(k - m) <= PAD).astype(np.float32) * scale

    bands = np.zeros((128, 4 * STRIDE), dtype=np.float32)
    # variant 0: first chunk (c=0), scale 1;  variant 1: first chunk, 81
    # variant 2: interior (c>=1), scale 1;    variant 3: interior, 81
    olo0, on0, ilo0, in0 = CHUNKS[0]
    bands[:in0, 0:on0] = band(ilo0, in0, olo0, on0, 1.0)
    bands[:in0, STRIDE:STRIDE + on0] = band(ilo0, in0, olo0, on0, 81.0)
    olo1, on1, ilo1, in1 = CHUNKS[1]
    bands[:in1, 2 * STRIDE:2 * STRIDE + on1] = band(ilo1, in1, olo1, on1, 1.0)
    bands[:in1, 3 * STRIDE:3 * STRIDE + on1] = band(ilo1, in1, olo1, on1, 81.0)
    bands_bf = bands.astype(ml_dtypes.bfloat16)
    ident_bf = np.eye(128, dtype=np.float32).astype(ml_dtypes.bfloat16)
    ones_f32 = np.ones((128, 1), dtype=np.float32)
    return bands_bf, bands, ident_bf, ones_f32


def _band_ap(bands_t, c, scaled):
    """AP into the packed bands tile for chunk c."""
    olo, on, ilo, inn = CHUNKS[c]
    if c == 0:
        v = 1 if scaled else 0
    else:
        v = 3 if scaled else 2
    return bands_t[0:inn, v * STRIDE:v * STRIDE + on]


def _build(nc):
    I_d = nc.dram_tensor("I", [H, W], F32, kind="ExternalInput").ap()
    J_d = nc.dram_tensor("J", [H, W], F32, kind="ExternalInput").ap()
    s0_d = nc.dram_tensor("s0", [H, W], F32, kind="ExternalInput").ap()
    s1_d = nc.dram_tensor("s1", [H, W], F32, kind="ExternalInput").ap()
    bands_d = nc.dram_tensor("bands", [128, 4 * STRIDE], BF16,
                             kind="ExternalInput").ap()
    bandsr_d = nc.dram_tensor("bandsr", [128, 4 * STRIDE], F32R,
                              kind="ExternalInput").ap()
    ident_d = nc.dram_tensor("ident", [128, 128], BF16,
                             kind="ExternalInput").ap()
    ones_d = nc.dram_tensor("ones", [128, 1], F32, kind="ExternalInput").ap()
    part_d = nc.dram_tensor("partials", [1, 68], F32,
                            kind="ExternalOutput").ap()

    from contextlib import ExitStack
    with tile.TileContext(nc) as tc, ExitStack() as ctx:
        consts = ctx.enter_context(tc.tile_pool(name="consts", bufs=1))
        inp = ctx.enter_context(tc.tile_pool(name="inp", bufs=2))
        prod = ctx.enter_context(tc.tile_pool(name="prod", bufs=2))
        smap = ctx.enter_context(tc.tile_pool(name="smap", bufs=1))
        tmap = ctx.enter_context(tc.tile_pool(name="tmap", bufs=2))
        ctmp = ctx.enter_context(tc.tile_pool(name="ctmp", bufs=2))
        spool = ctx.enter_context(tc.tile_pool(name="spool", bufs=2))
        accp = ctx.enter_context(tc.tile_pool(name="accp", bufs=1))
        psA = ctx.enter_context(tc.tile_pool(name="psA", bufs=2, space="PSUM"))
        ps2 = ctx.enter_context(tc.tile_pool(name="ps2", bufs=1, space="PSUM"))
        psF = ctx.enter_context(tc.tile_pool(name="psF", bufs=1, space="PSUM"))

        bands_t = consts.tile([128, 4 * STRIDE], BF16)
        bandsr_t = consts.tile([128, 4 * STRIDE], F32R)
        ident_t = consts.tile([128, 128], BF16)
        ones_t = consts.tile([128, 1], F32)
        nc.sync.dma_start(bands_t[:], bands_d)
        nc.sync.dma_start(bandsr_t[:], bandsr_d)
        nc.sync.dma_start(ident_t[:], ident_d)
        nc.sync.dma_start(ones_t[:], ones_d)

        # accumulators: accum_out OVERWRITES, so every accumulating
        # instruction gets its own column; host sums the groups.
        # cols 0-17: cc per (chunk,half); 18-33: lag_w; 34-49: lag_h;
        # 50-51: lag_h boundary; 52-67: s^2
        acc = accp.tile([128, 68], F32)
        nc.vector.memset(acc[:], 0.0)

        # ---------------- stage 1: H-conv -> S maps --------------------
        # S maps: per map 9 chunk tiles [out_n<=120, W] bf16, persistent
        MAPS = ("si", "sj", "sij", "sii", "sjj")
        s_tiles = {}
        for c, (olo, on, ilo, inn) in enumerate(CHUNKS):
            I_t = inp.tile([128, W], F32, tag="I_in")
            J_t = inp.tile([128, W], F32, tag="J_in")
            nc.sync.dma_start(I_t[0:inn, :], I_d[ilo:ilo + inn, :])
            nc.scalar.dma_start(J_t[0:inn, :], J_d[ilo:ilo + inn, :])

            sts = {}
            for name in MAPS:
                sts[name] = smap.tile([128, W], BF16, tag=f"S_{name}_{c}",
                                      name=f"S_{name}_{c}")
                s_tiles[(name, c)] = sts[name]
            for hw in range(2):
                wsl = slice(512 * hw, 512 * hw + 512)
                I_r = prod.tile([128, 512], F32R, tag="I_r")
                J_r = prod.tile([128, 512], F32R, tag="J_r")
                nc.vector.tensor_copy(I_r[0:inn, :], I_t[0:inn, wsl])
                nc.vector.tensor_copy(J_r[0:inn, :], J_t[0:inn, wsl])
                IJ_r = prod.tile([128, 512], F32R, tag="IJ_r")
                nc.vector.tensor_tensor(out=IJ_r[0:inn, :],
                                        in0=I_t[0:inn, wsl],
                                        in1=J_t[0:inn, wsl], op=ALU.mult)
                I2_r = prod.tile([128, 512], F32R, tag="I2_r")
                J2_r = prod.tile([128, 512], F32R, tag="J2_r")
                nc.scalar.square(I2_r[0:inn, :], I_t[0:inn, wsl])
                nc.scalar.square(J2_r[0:inn, :], J_t[0:inn, wsl])
                srcs = (I_r, J_r, IJ_r, I2_r, J2_r)
                for mi, name in enumerate(MAPS):
                    scaled = mi >= 2
                    p1 = psA.tile([128, 512], F32, tag="psA",
                                  padded_shape=[128, 512])
                    nc.tensor.matmul(p1[0:on, :],
                                     _band_ap(bandsr_t, c, scaled),
                                     srcs[mi][0:inn, :],
                                     start=True, stop=True)
                    if (c * 10 + mi * 2 + hw) % 2 == 0:
                        nc.vector.tensor_copy(sts[name][0:on, wsl],
                                              p1[0:on, :])
                    else:
                        nc.scalar.copy(sts[name][0:on, wsl], p1[0:on, :])

        # ------------- stage 2 per chunk: transpose, W-conv, combine ----
        for c2, (olo2, on2, ilo2, in2) in enumerate(CHUNKS):
            n = on2
            t_tiles = {}
            for mi, name in enumerate(MAPS):
                # transpose all 9 h'-segments into one bf16 psum bank
                pT = psA.tile([128, H], BF16, tag="psA", name="pT")
                for ch, (holo, hon, _, _) in enumerate(CHUNKS):
                    st = s_tiles[(name, ch)]
                    nc.tensor.matmul(
                        pT[0:in2, holo:holo + hon],
                        st[0:hon, ilo2:ilo2 + in2],
                        ident_t[0:hon, 0:hon],
                        is_transpose=True,
                        start=(ch == 0), stop=(ch == NCH - 1),
                        skip_group_check=True,
                    )
                tt = tmap.tile([128, H], BF16, tag=f"T_{name}")
                if (c2 * 5 + mi) % 2 == 0:
                    nc.vector.tensor_copy(tt[0:in2, :], pT[0:in2, :])
                else:
                    nc.scalar.copy(tt[0:in2, :], pT[0:in2, :])
                t_tiles[name] = tt

            for hw in range(2):
                hsl = slice(512 * hw, 512 * hw + 512)
                p2 = {}
                for mi, name in enumerate(MAPS):
                    p2[name] = ps2.tile([128, 512], F32, tag=f"p2_{name}", name=f"p2_{name}")
                    nc.tensor.matmul(p2[name][0:n, :],
                                     _band_ap(bands_t, c2, False),
                                     t_tiles[name][0:in2, hsl],
                                     start=True, stop=True)

                # combine in fp32, reading stage-2 psum directly
                # (max one PSUM operand per instruction)
                si_sb = ctmp.tile([128, 512], F32, tag="si_sb")
                nc.scalar.copy(si_sb[0:n, :], p2["si"][0:n, :])
                P = ctmp.tile([128, 512], F32, tag="P")
                nc.vector.tensor_tensor(out=P[0:n, :], in0=si_sb[0:n, :],
                                        in1=p2["sj"][0:n, :], op=ALU.mult)
                crossN = ctmp.tile([128, 512], F32, tag="crossN")
                nc.vector.tensor_tensor(out=crossN[0:n, :],
                                        in0=p2["sij"][0:n, :],
                                        in1=P[0:n, :], op=ALU.subtract)
                # PII = si^2 in-place over si_sb (si_sb dead afterwards)
                nc.scalar.square(si_sb[0:n, :], si_sb[0:n, :])
                IvarN = ctmp.tile([128, 512], F32, tag="IvarN")
                nc.vector.tensor_tensor(out=IvarN[0:n, :],
                                        in0=p2["sii"][0:n, :],
                                        in1=si_sb[0:n, :], op=ALU.subtract)
                PJJ = ctmp.tile([128, 512], F32, tag="PJJ")
                nc.scalar.square(PJJ[0:n, :], p2["sj"][0:n, :])
                JvarN = ctmp.tile([128, 512], F32, tag="JvarN")
                nc.vector.tensor_tensor(out=JvarN[0:n, :],
                                        in0=p2["sjj"][0:n, :],
                                        in1=PJJ[0:n, :], op=ALU.subtract)
                denom = ctmp.tile([128, 512], F32, tag="denom")
                nc.vector.tensor_tensor(out=denom[0:n, :], in0=IvarN[0:n, :],
                                        in1=JvarN[0:n, :], op=ALU.mult)
                # recip = exp(-ln(denom)), in-place
                nc.scalar.activation(denom[0:n, :], denom[0:n, :], ACTF.Ln)
                nc.scalar.activation(denom[0:n, :], denom[0:n, :], ACTF.Exp,
                                     scale=-1.0)
                # c2sq in-place over crossN
                nc.scalar.square(crossN[0:n, :], crossN[0:n, :])
                nc.vector.scalar_tensor_tensor(
                    out=crossN[0:n, :], in0=crossN[0:n, :], scalar=1.0,
                    in1=denom[0:n, :], op0=ALU.mult, op1=ALU.mult,
                    accum_out=acc[0:n, c2 * 2 + hw:c2 * 2 + hw + 1])

        # ---------------- smoothness over s0, s1 ------------------------
        for ch_i, s_d in enumerate((s0_d, s1_d)):
            for t in range(8):
                st = spool.tile([128, W], F32, tag="s_in")
                eng_d = nc.sync if t % 2 == 0 else nc.scalar
                eng_d.dma_start(st[:], s_d[128 * t:128 * (t + 1), :])
                # sum s^2 (output is junk; only the accumulator matters)
                s2o = spool.tile([128, W], F32, tag="junk")
                nc.scalar.activation(s2o[:], st[:], ACTF.Square,
                                     accum_out=acc[:, 52 + ch_i * 8 + t:
                                                   53 + ch_i * 8 + t])
                # lag_w: s[w]*s[w+1]
                lw = spool.tile([128, W], F32, tag="junk")
                nc.vector.scalar_tensor_tensor(
                    out=lw[:, 0:W - 1], in0=st[:, 1:W], scalar=1.0,
                    in1=st[:, 0:W - 1], op0=ALU.mult, op1=ALU.mult,
                    accum_out=acc[:, 18 + ch_i * 8 + t:19 + ch_i * 8 + t])
                # lag_h within tile: shift rows down via SBUF->SBUF DMA
                sh = spool.tile([128, W], F32, tag="sh")
                eng_d2 = nc.scalar if t % 2 == 0 else nc.sync
                eng_d2.dma_start(sh[0:127, :], st[1:128, :])
                lh = spool.tile([128, W], F32, tag="junk")
                nc.vector.scalar_tensor_tensor(
                    out=lh[0:127, :], in0=sh[0:127, :], scalar=1.0,
                    in1=st[0:127, :], op0=ALU.mult, op1=ALU.mult,
                    accum_out=acc[0:127, 34 + ch_i * 8 + t:
                                  35 + ch_i * 8 + t])

        # ---------------- final partition reduction ---------------------
        pF = psF.tile([1, 68], F32)
        nc.tensor.matmul(pF[:], ones_t[:], acc[:], start=True, stop=True)
        outt = accp.tile([1, 68], F32, tag="outt")
        nc.scalar.copy(outt[:], pF[:])
        nc.sync.dma_start(part_d, outt[:])

    return


def _get_nc():
    if "nc" not in _nc_cache:
        nc = bass.Bass("TRN2", target_bir_lowering=False, debug=False)
        _build(nc)
        _legalize_waits(nc)
        _nc_cache["nc"] = nc
    return _nc_cache["nc"]


def _const_map(consts):
    bands_bf, bands_f32, ident_bf, ones_f32 = consts
    return {
        "bands": bands_bf,
        "bandsr": bands_f32,
        "ident": ident_bf,
        "ones": ones_f32,
    }


def kernel(I, J, s, sum_filt):
    B = I.shape[0]
    assert I.shape == (B, 1, H, W) and s.shape == (B, 2, H, W)
    nc = _get_nc()
    consts = _make_host_consts()

    in_maps = []
    for b in range(B):
        m = {
            "I": np.ascontiguousarray(I[b, 0]),
            "J": np.ascontiguousarray(J[b, 0]),
            "s0": np.ascontiguousarray(s[b, 0]),
            "s1": np.ascontiguousarray(s[b, 1]),
        }
        m.update(_const_map(consts))
        in_maps.append(m)
    res = bass_utils.run_bass_kernel_spmd(nc, in_maps,
                                          core_ids=list(range(B)))
    parts = np.stack([res.results[b]["partials"][0] for b in range(B)])
    parts = parts.astype(np.float64)

    # host-side final assembly (float64)
    s64 = s.astype(np.float64)
    cc_sum = float(parts[:, 0:18].sum())
    lag_w = parts[:, 18:34].sum(axis=1)
    lag_h = parts[:, 34:52].sum(axis=1)
    s2 = parts[:, 52:68].sum(axis=1)

    # tile-boundary lag_h pairs (rows 127/128, 255/256, ...) per core
    rb = np.arange(127, H - 1, 128)
    lag_h = lag_h + (s64[:, :, rb, :] * s64[:, :, rb + 1, :]).sum(axis=(1, 2, 3))

    # edge corrections per core (both channels folded together)
    e_w = (s64[:, :, :, 0] ** 2).sum(axis=(1, 2)) + \
          (s64[:, :, :, -1] ** 2).sum(axis=(1, 2))
    e_h = (s64[:, :, 0, :] ** 2).sum(axis=(1, 2)) + \
          (s64[:, :, -1, :] ** 2).sum(axis=(1, 2))

    sum_dx2 = (2.0 * s2 - e_w - 2.0 * lag_w).sum()
    sum_dy2 = (2.0 * s2 - e_h - 2.0 * lag_h).sum()
    cnt = B * 2 * H * (W - 1)

    ncc_loss = -cc_sum / (B * H * W)
    smooth = 0.5 * (sum_dx2 / cnt + sum_dy2 / cnt) * ALPHA
    total = ncc_loss + smooth
    return np.array([total, ncc_loss, smooth], dtype=np.float32)



# revision 9
# speedup vs baseline: 1.5204x; 1.5204x over previous
"""Trainium2 Bass kernel for LocalCrossCorrelationWithSmoothnessLoss.

Full inputs in, full output out. Pure data-parallel over batch (B=8 -> 8
NeuronCores); each core computes partial sums for its image; the host
combines them into the three scalar losses.

Per-core pipeline (one 1024x1024 image pair + two flow channels):
  products  I16/J16 casts (GPSIMD), IJ (DVE), I2/J2 (ACT) -> f16 maps,
            10 row-chunks of <=128 rows (4-row conv halo baked in).
  pass 1    fused H-conv + transpose on the PE: stationary = data chunk
            [r_in, w 128], moving = banded box matrix [r_in, r' width]
            -> psum [w 128, r' 512-slice] f32.  Product maps use an
            81-scaled band.
  T-copy    psum -> SBUF f16 T maps [w 128, r' 1024] (DVE/ACT split).
  stage 2   W-conv: stationary = band [w 128, w' <=120], moving = T
            -> p2 psum [w', r' 512] f32 per map/half.
  combine   crossN = 81S_IJ - S_I*S_J, IvarN = 81S_II - S_I^2,
            JvarN = 81S_JJ - S_J^2, cc = (crossN * rsqrt(IvarN*JvarN))^2
            accumulated per-partition (DVE/ACT/GPSIMD split, bf16 temps).
  smooth    sum(s^2), lag_w = sum s[w]s[w+1] (DVE STT accum, fp32 2x);
            lag_h via PE shift-add matmul: t = s[p]+s[p+1] in psum, ACT
            Square-accum gives A = sum t^2; host recovers
            lag_h = (A - S - M)/2 from full (S) and partition-masked (M)
            column sums of s^2.  Tile-boundary rows fixed on the host.

Output per core: [2, 82] partial sums (row 1 = partition-0-masked).
Host assembles the losses in float64.
"""
import sys
import numpy as np

sys.path.insert(0, "/opt/trn_rl_repo")

import ml_dtypes
import bass_rust
import concourse.bass as bass
import concourse.tile as tile
from concourse import mybir
from concourse import bass_utils
from concourse import tile_utils

F32 = mybir.dt.float32
F32R = mybir.dt.float32r
F16 = mybir.dt.float16
BF16 = mybir.dt.bfloat16
ALU = mybir.AluOpType
ACTF = mybir.ActivationFunctionType

H = 1024
W = 1024
PAD = 4
WIN = 81.0
ALPHA = 0.01

# r'-chunks for pass-1 (out range, in range). 512-aligned slices:
# {120,120,120,120,32} x 2.  in = out +- PAD clamped to [0, H].
RCH = []
for _lo in (0, 120, 240, 360, 480, 512, 632, 752, 872, 992):
    _n = 32 if _lo in (480, 992) else 120
    _ilo = max(0, _lo - PAD)
    _ihi = min(H, _lo + _n + PAD)
    RCH.append((_lo, _n, _ilo, _ihi - _ilo))
NRC = len(RCH)

# w'-chunks for stage-2: out w' range + the 128-wide stationary col window.
WCH = []
for _j in range(9):
    _olo = 120 * _j
    _on = min(120, W - _olo)
    _clo = 0 if _j == 0 else (W - 128 if _olo + _on + PAD > W else _olo - PAD)
    WCH.append((_olo, _on, _clo))
NWC = len(WCH)

MAPS = ("si", "sj", "sij", "sii", "sjj")

# accumulator columns
ACC_CC = 0          # 18: (j, half)
ACC_S2 = 18         # 16: (ch, tile)
ACC_LW = 34         # 16
ACC_SH = 50         # 32: (ch, tile, half)
NACC = 82

tile_utils.max_sbuf_usage = 206 * 1024

_nc_cache = {}


def _legalize_waits(nc, max_waits=1):
    """walrus accepts only one sync-wait per instruction; split extras
    onto same-engine NoOps placed just before."""
    ctr = 0
    for f in nc.m.functions:
        for bb in f.blocks:
            insts = bb.instructions
            i = 0
            while i < len(insts):
                ins = insts[i]
                si = ins.sync_info
                if si is None:
                    i += 1
                    continue
                w = list(si.on_wait)
                if len(w) <= max_waits:
                    i += 1
                    continue
                extra, keep = w[:-max_waits], w[-max_waits:]
                nops = []
                for j in range(0, len(extra), max_waits):
                    chunk = extra[j:j + max_waits]
                    nop = mybir.InstNoOp(name=f"I-wsplit-{ctr}", ins=[], outs=[])
                    ctr += 1
                    nop.engine = ins.engine
                    nop.sync_info = bass_rust.SyncInfo(on_wait=chunk, on_update=[])
                    nops.append(nop)
                ins.sync_info = bass_rust.SyncInfo(on_wait=keep,
                                                  on_update=list(si.on_update))
                insts[i:i] = nops
                i += len(nops) + 1


def _band(klo, kn, olo, on, scale):
    k = np.arange(klo, klo + kn)[:, None]
    m = np.arange(olo, olo + on)[None, :]
    return (np.abs(k - m) <= PAD).astype(np.float32) * scale


def _make_host_consts():
    # bands tile [128, 544] f16:
    #   cols   0:120  B0    = |k - m|     <= 4   (unscaled)
    #   cols 120:240  Bmid  = |k - 4 - m| <= 4   (unscaled)
    #   cols 240:304  B8    = |k - 64 - m| <= 4  (unscaled, stage-2 j=8)
    #   cols 304:424  B0s   = B0 * 81
    #   cols 424:544  Bmids = Bmid * 81
    bands = np.zeros((128, 544), dtype=np.float32)
    bands[:, 0:120] = _band(0, 128, 0, 120, 1.0)
    bands[:, 120:240] = _band(0, 128, 4, 120, 1.0)
    bands[:, 240:304] = _band(0, 128, 64, 64, 1.0)
    bands[:, 304:424] = _band(0, 128, 0, 120, WIN)
    bands[:, 424:544] = _band(0, 128, 4, 120, WIN)
    bands_f16 = bands.astype(np.float16)

    # shift-add matrix [128, 128] f32: out[p] = s[p] + s[p+1]
    sadd = np.zeros((128, 128), dtype=np.float32)
    for p in range(128):
        sadd[p, p] = 1.0
        if p + 1 < 128:
            sadd[p + 1, p] = 1.0

    # ones [128, 2]: col 0 full, col 1 masks partition 0
    onesp = np.ones((128, 2), dtype=np.float32)
    onesp[0, 1] = 0.0
    return bands_f16, sadd, onesp


def _const_map(consts):
    bands_f16, sadd, onesp = consts
    return {"bands": bands_f16, "sadd": sadd, "onesp": onesp}


def _band_r(bands_t, c, scaled):
    """Moving band AP for pass-1 r-chunk c: [r_in rows, out cols]."""
    olo, on, ilo, inn = RCH[c]
    if c == 0:
        base = 304 if scaled else 0
    else:
        base = 424 if scaled else 120
    return bands_t[0:inn, base:base + on]


def _band_w(bands_t, j):
    """Stationary band AP for stage-2 w-chunk j: [128, out cols]."""
    olo, on, clo = WCH[j]
    if j == 0:
        return bands_t[0:128, 0:on]
    if olo - PAD == clo:
        return bands_t[0:128, 120:120 + on]
    return bands_t[0:128, 240:240 + on]


def _build(nc):
    I_d = nc.dram_tensor("I", [H, W], F32, kind="ExternalInput").ap()
    J_d = nc.dram_tensor("J", [H, W], F32, kind="ExternalInput").ap()
    s0_d = nc.dram_tensor("s0", [H, W], F32R, kind="ExternalInput").ap()
    s1_d = nc.dram_tensor("s1", [H, W], F32R, kind="ExternalInput").ap()
    bands_d = nc.dram_tensor("bands", [128, 544], F16,
                             kind="ExternalInput").ap()
    sadd_d = nc.dram_tensor("sadd", [128, 128], F32R,
                            kind="ExternalInput").ap()
    onesp_d = nc.dram_tensor("onesp", [128, 2], F32,
                             kind="ExternalInput").ap()
    part_d = nc.dram_tensor("partials", [2, NACC], F32,
                            kind="ExternalOutput").ap()

    from contextlib import ExitStack
    with tile.TileContext(nc) as tc, ExitStack() as ctx:
        consts = ctx.enter_context(tc.tile_pool(name="consts", bufs=1))
        inp = ctx.enter_context(tc.tile_pool(name="inp", bufs=2))
        xmap = ctx.enter_context(tc.tile_pool(name="xmap", bufs=1))
        tmap = ctx.enter_context(tc.tile_pool(name="tmap", bufs=2))
        ctmp = ctx.enter_context(tc.tile_pool(name="ctmp", bufs=2))
        spool = ctx.enter_context(tc.tile_pool(name="spool", bufs=3))
        sjunk = ctx.enter_context(tc.tile_pool(name="sjunk", bufs=2))
        accp = ctx.enter_context(tc.tile_pool(name="accp", bufs=1))
        psT = ctx.enter_context(tc.tile_pool(name="psT", bufs=3, space="PSUM"))
        ps2 = ctx.enter_context(tc.tile_pool(name="ps2", bufs=3, space="PSUM"))
        psS = ctx.enter_context(tc.tile_pool(name="psS", bufs=1, space="PSUM"))
        psF = ctx.enter_context(tc.tile_pool(name="psF", bufs=1, space="PSUM"))

        bands_t = consts.tile([128, 544], F16)
        sadd_t = consts.tile([128, 128], F32R)
        onesp_t = consts.tile([128, 2], F32)
        nc.scalar.dma_start(bands_t[:], bands_d)
        nc.scalar.dma_start(sadd_t[:], sadd_d)
        nc.scalar.dma_start(onesp_t[:], onesp_d)

        acc = accp.tile([128, NACC], F32)
        nc.vector.memset(acc[:], 0.0)

        # ---------------- emission helpers --------------------------------
        def load_rows(dst, src, r0, n, nslices):
            """sliced HBM load on the SP queue."""
            step = (n + nslices - 1) // nslices
            o = 0
            while o < n:
                m = min(step, n - o)
                nc.sync.dma_start(dst[o:o + m, :], src[r0 + o:r0 + o + m, :])
                o += m

        s_tiles_done = [0]

        def emit_s_tile():
            """one smoothness tile: load, s^2, lag_w, shift-add + A."""
            k = s_tiles_done[0]
            if k >= 16:
                return
            s_tiles_done[0] += 1
            ch, t = k // 8, k % 8
            s_d = s0_d if ch == 0 else s1_d
            st = spool.tile([128, W], F32R, tag="s_in")
            load_rows(st, s_d, 128 * t, 128, 2)
            # s^2 and lag_w on DVE (fp32 all-SBUF STT -> 2x mode)
            o1 = sjunk.tile([128, W], F32, tag="so1")
            nc.vector.scalar_tensor_tensor(
                out=o1[:], in0=st[:], scalar=1.0, in1=st[:],
                op0=ALU.mult, op1=ALU.mult,
                accum_out=acc[:, ACC_S2 + k:ACC_S2 + k + 1])
            o2 = sjunk.tile([128, W], F32, tag="so2")
            nc.vector.scalar_tensor_tensor(
                out=o2[:, 0:W - 1], in0=st[:, 1:W], scalar=1.0,
                in1=st[:, 0:W - 1], op0=ALU.mult, op1=ALU.mult,
                accum_out=acc[:, ACC_LW + k:ACC_LW + k + 1])
            # lag_h: t = s[p] + s[p+1] via PE, A = sum t^2 via ACT
            for hh in range(2):
                hsl = slice(512 * hh, 512 * hh + 512)
                pS = psS.tile([128, 512], F32, tag="psS")
                nc.tensor.matmul(pS[:, :], sadd_t[:], st[:, hsl],
                                 start=True, stop=True)
                o3 = sjunk.tile([128, 512], F32, tag="so3")
                col = ACC_SH + 2 * k + hh
                nc.scalar.activation(o3[:], pS[:, :], ACTF.Square,
                                     accum_out=acc[:, col:col + 1])

        # ---------------- products (10 r-chunks) --------------------------
        x_tiles = {}
        for c, (olo, on, ilo, inn) in enumerate(RCH):
            I_t = inp.tile([128, W], F32, tag="I_in")
            J_t = inp.tile([128, W], F32, tag="J_in")
            load_rows(I_t, I_d, ilo, inn, 2)
            load_rows(J_t, J_d, ilo, inn, 2)
            for name in MAPS:
                x_tiles[(name, c)] = xmap.tile([128, W], F16,
                                               tag=f"X_{name}_{c}",
                                               name=f"X_{name}_{c}")
            # casts on GPSIMD, squares on ACT, cross product on DVE
            nc.gpsimd.tensor_copy(x_tiles[("si", c)][0:inn, :], I_t[0:inn, :])
            nc.gpsimd.tensor_copy(x_tiles[("sj", c)][0:inn, :], J_t[0:inn, :])
            nc.scalar.square(x_tiles[("sii", c)][0:inn, :], I_t[0:inn, :])
            nc.scalar.square(x_tiles[("sjj", c)][0:inn, :], J_t[0:inn, :])
            nc.vector.tensor_tensor(out=x_tiles[("sij", c)][0:inn, :],
                                    in0=I_t[0:inn, :], in1=J_t[0:inn, :],
                                    op=ALU.mult)
            if c % 2 == 1:
                emit_s_tile()

        # ---------------- main loop over w-chunks --------------------------
        for j, (wolo, won, wclo) in enumerate(WCH):
            wsl = slice(wclo, wclo + 128)
            t_tiles = {}
            # pass 1: fused H-conv + transpose into psum
            for mi, name in enumerate(MAPS):
                scaled = mi >= 2
                t_tiles[name] = tmap.tile([128, W], F16, tag=f"T_{name}",
                                          name=f"T_{name}_{j}")
                for hh in range(2):
                    pT = psT.tile([128, 512], F32, tag="psT")
                    cs = list(range(5 * hh, 5 * hh + 5))
                    for c in cs:
                        olo, on, ilo, inn = RCH[c]
                        nc.tensor.matmul(
                            pT[0:128, olo - 512 * hh:olo - 512 * hh + on],
                            x_tiles[(name, c)][0:inn, wsl],
                            _band_r(bands_t, c, scaled),
                            start=(c == cs[0]), stop=(c == cs[-1]),
                            skip_group_check=True)
                    # T-copy psum -> SBUF f16, rotate DVE/ACT
                    dst = t_tiles[name][0:128, 512 * hh:512 * hh + 512]
                    if (j * 10 + mi * 2 + hh) % 2 == 0:
                        nc.vector.tensor_copy(dst, pT[:, :])
                    else:
                        nc.scalar.copy(dst, pT[:, :])

            # stage 2: W-conv
            p2 = {}
            for name in MAPS:
                for hh in range(2):
                    p = ps2.tile([128, 512], F32, tag="p2")
                    nc.tensor.matmul(p[0:won, :], _band_w(bands_t, j),
                                     t_tiles[name][0:128,
                                                   512 * hh:512 * hh + 512],
                                     start=True, stop=True)
                    p2[(name, hh)] = p

            # combine per half
            for hh in range(2):
                n = won
                si_p, sj_p = p2[("si", hh)], p2[("sj", hh)]
                sij_p, sii_p, sjj_p = (p2[("sij", hh)], p2[("sii", hh)],
                                       p2[("sjj", hh)])
                si_sb = ctmp.tile([128, 512], BF16, tag="si_sb")
                nc.scalar.copy(si_sb[0:n, :], si_p[0:n, :])
                P = ctmp.tile([128, 512], BF16, tag="P")
                nc.vector.tensor_tensor(out=P[0:n, :], in0=si_sb[0:n, :],
                                        in1=sj_p[0:n, :], op=ALU.mult)
                crossN = ctmp.tile([128, 512], BF16, tag="crossN")
                nc.vector.tensor_tensor(out=crossN[0:n, :],
                                        in0=sij_p[0:n, :], in1=P[0:n, :],
                                        op=ALU.subtract)
                si2 = ctmp.tile([128, 512], BF16, tag="si2")
                nc.gpsimd.tensor_tensor(out=si2[0:n, :], in0=si_sb[0:n, :],
                                        in1=si_sb[0:n, :], op=ALU.mult)
                IvarN = ctmp.tile([128, 512], BF16, tag="IvarN")
                nc.vector.tensor_tensor(out=IvarN[0:n, :], in0=sii_p[0:n, :],
                                        in1=si2[0:n, :], op=ALU.subtract)
                sj2 = ctmp.tile([128, 512], BF16, tag="sj2")
                nc.scalar.square(sj2[0:n, :], sj_p[0:n, :])
                JvarN = ctmp.tile([128, 512], BF16, tag="JvarN")
                nc.vector.tensor_tensor(out=JvarN[0:n, :], in0=sjj_p[0:n, :],
                                        in1=sj2[0:n, :], op=ALU.subtract)
                denom = ctmp.tile([128, 512], F32, tag="denom")
                nc.gpsimd.tensor_tensor(out=denom[0:n, :], in0=IvarN[0:n, :],
                                        in1=JvarN[0:n, :], op=ALU.mult)
                # recip = exp(-ln(denom)); ln/exp/square/copy share one
                # ACT table so no table reloads
                nc.scalar.activation(denom[0:n, :], denom[0:n, :], ACTF.Ln)
                nc.scalar.activation(denom[0:n, :], denom[0:n, :], ACTF.Exp,
                                     scale=-1.0)
                c2 = ctmp.tile([128, 512], BF16, tag="c2")
                nc.vector.scalar_tensor_tensor(
                    out=c2[0:n, :], in0=crossN[0:n, :], scalar=1.0,
                    in1=crossN[0:n, :], op0=ALU.mult, op1=ALU.mult)
                qj = ctmp.tile([128, 512], BF16, tag="qj")
                col = ACC_CC + 2 * j + hh
                nc.vector.scalar_tensor_tensor(
                    out=qj[0:n, :], in0=c2[0:n, :], scalar=1.0,
                    in1=denom[0:n, :], op0=ALU.mult, op1=ALU.mult,
                    accum_out=acc[0:n, col:col + 1])

            emit_s_tile()

        while s_tiles_done[0] < 16:
            emit_s_tile()

        # ---------------- final partition reduction ------------------------
        pF = psF.tile([2, NACC], F32, tag="pF")
        nc.tensor.matmul(pF[:], onesp_t[:], acc[:], start=True, stop=True)
        outt = accp.tile([2, NACC], F32, tag="outt")
        nc.scalar.copy(outt[:], pF[:])
        nc.scalar.dma_start(part_d, outt[:])

    return


def _get_nc():
    if "nc" not in _nc_cache:
        nc = bass.Bass("TRN2", target_bir_lowering=False, debug=False)
        _build(nc)
        _legalize_waits(nc)
        _nc_cache["nc"] = nc
    return _nc_cache["nc"]


def kernel(I, J, s, sum_filt):
    B = I.shape[0]
    assert I.shape == (B, 1, H, W) and s.shape == (B, 2, H, W)
    nc = _get_nc()
    consts = _make_host_consts()

    in_maps = []
    for b in range(B):
        m = {
            "I": np.ascontiguousarray(I[b, 0]),
            "J": np.ascontiguousarray(J[b, 0]),
            "s0": np.ascontiguousarray(s[b, 0]),
            "s1": np.ascontiguousarray(s[b, 1]),
        }
        m.update(_const_map(consts))
        in_maps.append(m)
    res = bass_utils.run_bass_kernel_spmd(nc, in_maps,
                                          core_ids=list(range(B)))
    parts = np.stack([res.results[b]["partials"] for b in range(B)])
    parts = parts.astype(np.float64)  # [B, 2, NACC]

    s64 = s.astype(np.float64)
    cc_sum = float(parts[:, 0, ACC_CC:ACC_CC + 18].sum())
    s2_full = parts[:, 0, ACC_S2:ACC_S2 + 16]        # [B, 16]
    s2_mask = parts[:, 1, ACC_S2:ACC_S2 + 16]
    lag_w = parts[:, 0, ACC_LW:ACC_LW + 16].sum(axis=1)
    A = (parts[:, 0, ACC_SH::2] + parts[:, 0, ACC_SH + 1::2])  # [B, 16]
    lag_h = ((A - s2_full - s2_mask) / 2.0).sum(axis=1)
    s2 = s2_full.sum(axis=1)

    # tile-boundary lag_h pairs (rows 127/128, ...) per core
    rb = np.arange(127, H - 1, 128)
    lag_h = lag_h + (s64[:, :, rb, :] * s64[:, :, rb + 1, :]).sum(axis=(1, 2, 3))

    # edge corrections per core (both channels folded together)
    e_w = (s64[:, :, :, 0] ** 2).sum(axis=(1, 2)) + \
          (s64[:, :, :, -1] ** 2).sum(axis=(1, 2))
    e_h = (s64[:, :, 0, :] ** 2).sum(axis=(1, 2)) + \
          (s64[:, :, -1, :] ** 2).sum(axis=(1, 2))

    sum_dx2 = (2.0 * s2 - e_w - 2.0 * lag_w).sum()
    sum_dy2 = (2.0 * s2 - e_h - 2.0 * lag_h).sum()
    cnt = B * 2 * H * (W - 1)

    ncc_loss = -cc_sum / (B * H * W)
    smooth = 0.5 * (sum_dx2 / cnt + sum_dy2 / cnt) * ALPHA
    total = ncc_loss + smooth
    return np.array([total, ncc_loss, smooth], dtype=np.float32)


# revision 10
# speedup vs baseline: 1.9900x; 1.3088x over previous
"""Trainium2 Bass kernel for LocalCrossCorrelationWithSmoothnessLoss.

Full inputs in, full output out. Pure data-parallel over batch (B=8 -> 8
NeuronCores); each core computes partial sums for its image; the host
combines them into the three scalar losses.

Per-core pipeline (one 1024x1024 image pair + two flow channels):
  products  I16/J16 casts (GPSIMD), IJ (DVE), I2/J2 (ACT) -> f16 maps,
            10 row-chunks of <=128 rows (4-row conv halo baked in).
  pass 1    fused H-conv + transpose on the PE: stationary = data chunk
            [r_in, w 128], moving = banded box matrix [r_in, r' width]
            -> psum [w 128, r' 512-slice] f32.  Product maps use an
            81-scaled band.
  T-copy    psum -> SBUF f16 T maps [w 128, r' 1024] (DVE/ACT split).
  stage 2   W-conv: stationary = band [w 128, w' <=120], moving = T
            -> p2 psum [w', r' 512] f32 per map/half.
  combine   crossN = 81S_IJ - S_I*S_J, IvarN = 81S_II - S_I^2,
            JvarN = 81S_JJ - S_J^2, cc = (crossN * rsqrt(IvarN*JvarN))^2
            accumulated per-partition (DVE/ACT/GPSIMD split, bf16 temps).
  smooth    sum(s^2), lag_w = sum s[w]s[w+1] (DVE STT accum, fp32 2x);
            lag_h via PE shift-add matmul: t = s[p]+s[p+1] in psum, ACT
            Square-accum gives A = sum t^2; host recovers
            lag_h = (A - S - M)/2 from full (S) and partition-masked (M)
            column sums of s^2.  Tile-boundary rows fixed on the host.

Output per core: [2, 82] partial sums (row 1 = partition-0-masked).
Host assembles the losses in float64.
"""
import sys
import numpy as np

sys.path.insert(0, "/opt/trn_rl_repo")

import ml_dtypes
import bass_rust
import concourse.bass as bass
import concourse.tile as tile
from concourse import mybir
from concourse import bass_utils
from concourse import tile_utils

F32 = mybir.dt.float32
F32R = mybir.dt.float32r
F16 = mybir.dt.float16
BF16 = mybir.dt.bfloat16
ALU = mybir.AluOpType
ACTF = mybir.ActivationFunctionType

H = 1024
W = 1024
PAD = 4
WIN = 81.0
ALPHA = 0.01

# r'-chunks for pass-1 (out range, in range). 512-aligned slices:
# {120,120,120,120,32} x 2.  in = out +- PAD clamped to [0, H].
RCH = []
for _lo in (0, 120, 240, 360, 480, 512, 632, 752, 872, 992):
    _n = 32 if _lo in (480, 992) else 120
    _ilo = max(0, _lo - PAD)
    _ihi = min(H, _lo + _n + PAD)
    RCH.append((_lo, _n, _ilo, _ihi - _ilo))
NRC = len(RCH)

# w'-chunks for stage-2: out w' range + the 128-wide stationary col window.
WCH = []
for _j in range(9):
    _olo = 120 * _j
    _on = min(120, W - _olo)
    _clo = 0 if _j == 0 else (W - 128 if _olo + _on + PAD > W else _olo - PAD)
    WCH.append((_olo, _on, _clo))
NWC = len(WCH)

MAPS = ("si", "sj", "sij", "sii", "sjj")

# accumulator columns
ACC_CC = 0          # 18: (j, half)
ACC_S2 = 18         # 16: (ch, tile)
ACC_LW = 34         # 16
ACC_SH = 50         # 32: (ch, tile, half)
NACC = 82

tile_utils.max_sbuf_usage = 206 * 1024

_nc_cache = {}


def _legalize_waits(nc, max_waits=1):
    """walrus accepts only one sync-wait per instruction; split extras
    onto same-engine NoOps placed just before."""
    ctr = 0
    for f in nc.m.functions:
        for bb in f.blocks:
            insts = bb.instructions
            i = 0
            while i < len(insts):
                ins = insts[i]
                si = ins.sync_info
                if si is None:
                    i += 1
                    continue
                w = list(si.on_wait)
                if len(w) <= max_waits:
                    i += 1
                    continue
                extra, keep = w[:-max_waits], w[-max_waits:]
                nops = []
                for j in range(0, len(extra), max_waits):
                    chunk = extra[j:j + max_waits]
                    nop = mybir.InstNoOp(name=f"I-wsplit-{ctr}", ins=[], outs=[])
                    ctr += 1
                    nop.engine = ins.engine
                    nop.sync_info = bass_rust.SyncInfo(on_wait=chunk, on_update=[])
                    nops.append(nop)
                ins.sync_info = bass_rust.SyncInfo(on_wait=keep,
                                                  on_update=list(si.on_update))
                insts[i:i] = nops
                i += len(nops) + 1


def _act_raw(nc, out, in_, func, scale=1.0, accum_out=None):
    """InstActivation without the bass Rsqrt/Reciprocal guard."""
    se = nc.scalar
    bias = nc.const_aps.scalar_like(0.0, in_)
    ins = [se.lower_ap(in_), se.lower_ap(bias),
           mybir.ImmediateValue(dtype=mybir.dt.float32, value=scale),
           mybir.ImmediateValue(dtype=mybir.dt.float32, value=0.0)]
    outs = [se.lower_ap(out)]
    if accum_out is not None:
        outs.append(se.lower_ap(accum_out))
    return se.add_instruction(mybir.InstActivation(
        name=nc.get_next_instruction_name(), func=func, ins=ins, outs=outs))


def _band(klo, kn, olo, on, scale):
    k = np.arange(klo, klo + kn)[:, None]
    m = np.arange(olo, olo + on)[None, :]
    return (np.abs(k - m) <= PAD).astype(np.float32) * scale


def _make_host_consts():
    # bands tile [128, 544] f16:
    #   cols   0:120  B0    = |k - m|     <= 4   (unscaled)
    #   cols 120:240  Bmid  = |k - 4 - m| <= 4   (unscaled)
    #   cols 240:304  B8    = |k - 64 - m| <= 4  (unscaled, stage-2 j=8)
    #   cols 304:424  B0s   = B0 * 81
    #   cols 424:544  Bmids = Bmid * 81
    bands = np.zeros((128, 544), dtype=np.float32)
    bands[:, 0:120] = _band(0, 128, 0, 120, 1.0)
    bands[:, 120:240] = _band(0, 128, 4, 120, 1.0)
    bands[:, 240:304] = _band(0, 128, 64, 64, 1.0)
    bands[:, 304:424] = _band(0, 128, 0, 120, WIN)
    bands[:, 424:544] = _band(0, 128, 4, 120, WIN)
    bands_f16 = bands.astype(np.float16)

    # shift-add matrix [128, 128] f32: out[p] = s[p] + s[p+1]
    sadd = np.zeros((128, 128), dtype=np.float32)
    for p in range(128):
        sadd[p, p] = 1.0
        if p + 1 < 128:
            sadd[p + 1, p] = 1.0

    # ones [128, 2]: col 0 full, col 1 masks partition 0
    onesp = np.ones((128, 2), dtype=np.float32)
    onesp[0, 1] = 0.0
    return bands_f16, sadd, onesp


def _const_map(consts):
    bands_f16, sadd, onesp = consts
    return {"bands": bands_f16, "sadd": sadd, "onesp": onesp}


def _band_r(bands_t, c, scaled):
    """Moving band AP for pass-1 r-chunk c: [r_in rows, out cols]."""
    olo, on, ilo, inn = RCH[c]
    if c == 0:
        base = 304 if scaled else 0
    else:
        base = 424 if scaled else 120
    return bands_t[0:inn, base:base + on]


def _band_w(bands_t, j):
    """Stationary band AP for stage-2 w-chunk j: [128, out cols]."""
    olo, on, clo = WCH[j]
    if j == 0:
        return bands_t[0:128, 0:on]
    if olo - PAD == clo:
        return bands_t[0:128, 120:120 + on]
    return bands_t[0:128, 240:240 + on]


def _build(nc):
    I_d = nc.dram_tensor("I", [H, W], F32, kind="ExternalInput").ap()
    J_d = nc.dram_tensor("J", [H, W], F32, kind="ExternalInput").ap()
    s0_d = nc.dram_tensor("s0", [H, W], F32R, kind="ExternalInput").ap()
    s1_d = nc.dram_tensor("s1", [H, W], F32R, kind="ExternalInput").ap()
    bands_d = nc.dram_tensor("bands", [128, 544], F16,
                             kind="ExternalInput").ap()
    sadd_d = nc.dram_tensor("sadd", [128, 128], F32R,
                            kind="ExternalInput").ap()
    onesp_d = nc.dram_tensor("onesp", [128, 2], F32,
                             kind="ExternalInput").ap()
    part_d = nc.dram_tensor("partials", [2, NACC], F32,
                            kind="ExternalOutput").ap()

    from contextlib import ExitStack
    with tile.TileContext(nc) as tc, ExitStack() as ctx:
        consts = ctx.enter_context(tc.tile_pool(name="consts", bufs=1))
        inp = ctx.enter_context(tc.tile_pool(name="inp", bufs=2))
        xmap = ctx.enter_context(tc.tile_pool(name="xmap", bufs=1))
        tmap = ctx.enter_context(tc.tile_pool(name="tmap", bufs=2))
        ctmp = ctx.enter_context(tc.tile_pool(name="ctmp", bufs=2))
        spool = ctx.enter_context(tc.tile_pool(name="spool", bufs=3))
        sjunk = ctx.enter_context(tc.tile_pool(name="sjunk", bufs=2))
        accp = ctx.enter_context(tc.tile_pool(name="accp", bufs=1))
        psT = ctx.enter_context(tc.tile_pool(name="psT", bufs=3, space="PSUM"))
        ps2 = ctx.enter_context(tc.tile_pool(name="ps2", bufs=3, space="PSUM"))
        psS = ctx.enter_context(tc.tile_pool(name="psS", bufs=1, space="PSUM"))
        psF = ctx.enter_context(tc.tile_pool(name="psF", bufs=1, space="PSUM"))

        bands_t = consts.tile([128, 544], F16)
        sadd_t = consts.tile([128, 128], F32R)
        onesp_t = consts.tile([128, 2], F32)
        nc.scalar.dma_start(bands_t[:], bands_d)
        nc.scalar.dma_start(sadd_t[:], sadd_d)
        nc.scalar.dma_start(onesp_t[:], onesp_d)

        acc = accp.tile([128, NACC], F32)
        nc.vector.memset(acc[:], 0.0)

        # ---------------- emission helpers --------------------------------
        def load_rows(dst, src, r0, n, nslices, eng=None):
            """sliced HBM load on a HWDGE queue (sync or scalar)."""
            eng = eng or nc.sync
            step = (n + nslices - 1) // nslices
            o = 0
            while o < n:
                m = min(step, n - o)
                eng.dma_start(dst[o:o + m, :], src[r0 + o:r0 + o + m, :])
                o += m

        s_tiles_done = [0]

        def emit_s_tile():
            """one smoothness tile: load, s^2, lag_w, shift-add + A."""
            k = s_tiles_done[0]
            if k >= 16:
                return
            s_tiles_done[0] += 1
            ch, t = k // 8, k % 8
            s_d = s0_d if ch == 0 else s1_d
            st = spool.tile([128, W], F32R, tag="s_in")
            load_rows(st, s_d, 128 * t, 128, 2,
                      eng=(nc.sync if k % 2 == 0 else nc.scalar))
            # s^2 and lag_w on DVE (fp32 all-SBUF STT -> 2x mode)
            o1 = sjunk.tile([128, W], F32, tag="so1")
            nc.vector.scalar_tensor_tensor(
                out=o1[:], in0=st[:], scalar=1.0, in1=st[:],
                op0=ALU.mult, op1=ALU.mult,
                accum_out=acc[:, ACC_S2 + k:ACC_S2 + k + 1])
            o2 = sjunk.tile([128, W], F32, tag="so2")
            nc.vector.scalar_tensor_tensor(
                out=o2[:, 0:W - 1], in0=st[:, 1:W], scalar=1.0,
                in1=st[:, 0:W - 1], op0=ALU.mult, op1=ALU.mult,
                accum_out=acc[:, ACC_LW + k:ACC_LW + k + 1])
            # lag_h: t = s[p] + s[p+1] via PE, A = sum t^2 via ACT
            for hh in range(2):
                hsl = slice(512 * hh, 512 * hh + 512)
                pS = psS.tile([128, 512], F32, tag="psS")
                nc.tensor.matmul(pS[:, :], sadd_t[:], st[:, hsl],
                                 start=True, stop=True)
                o3 = sjunk.tile([128, 512], F32, tag="so3")
                col = ACC_SH + 2 * k + hh
                nc.scalar.activation(o3[:], pS[:, :], ACTF.Square,
                                     accum_out=acc[:, col:col + 1])

        # ---------------- products (10 r-chunks) --------------------------
        x_tiles = {}
        for c, (olo, on, ilo, inn) in enumerate(RCH):
            I_t = inp.tile([128, W], F32, tag="I_in")
            J_t = inp.tile([128, W], F32, tag="J_in")
            load_rows(I_t, I_d, ilo, inn, 2)
            load_rows(J_t, J_d, ilo, inn, 2, eng=nc.scalar)
            for name in MAPS:
                x_tiles[(name, c)] = xmap.tile([128, W], F16,
                                               tag=f"X_{name}_{c}",
                                               name=f"X_{name}_{c}")
            # casts on GPSIMD, squares on ACT, cross product on DVE
            if c % 2 == 0:
                nc.gpsimd.tensor_copy(x_tiles[("si", c)][0:inn, :],
                                      I_t[0:inn, :])
                nc.gpsimd.tensor_copy(x_tiles[("sj", c)][0:inn, :],
                                      J_t[0:inn, :])
            else:
                nc.vector.tensor_copy(x_tiles[("si", c)][0:inn, :],
                                      I_t[0:inn, :])
                nc.scalar.copy(x_tiles[("sj", c)][0:inn, :], J_t[0:inn, :])
            nc.scalar.square(x_tiles[("sii", c)][0:inn, :], I_t[0:inn, :])
            nc.scalar.square(x_tiles[("sjj", c)][0:inn, :], J_t[0:inn, :])
            nc.vector.tensor_tensor(out=x_tiles[("sij", c)][0:inn, :],
                                    in0=I_t[0:inn, :], in1=J_t[0:inn, :],
                                    op=ALU.mult)
            if c % 2 == 1:
                emit_s_tile()

        # ---------------- main loop over w-chunks --------------------------
        for j, (wolo, won, wclo) in enumerate(WCH):
            wsl = slice(wclo, wclo + 128)
            t_tiles = {}
            # pass 1: fused H-conv + transpose into psum
            for mi, name in enumerate(MAPS):
                scaled = mi >= 2
                t_tiles[name] = tmap.tile([128, W], F16, tag=f"T_{name}",
                                          name=f"T_{name}_{j}")
                for hh in range(2):
                    pT = psT.tile([128, 512], F32, tag="psT")
                    cs = list(range(5 * hh, 5 * hh + 5))
                    for c in cs:
                        olo, on, ilo, inn = RCH[c]
                        nc.tensor.matmul(
                            pT[0:128, olo - 512 * hh:olo - 512 * hh + on],
                            x_tiles[(name, c)][0:inn, wsl],
                            _band_r(bands_t, c, scaled),
                            start=(c == cs[0]), stop=(c == cs[-1]),
                            skip_group_check=True)
                    # T-copy psum -> SBUF f16, rotate DVE/ACT
                    dst = t_tiles[name][0:128, 512 * hh:512 * hh + 512]
                    if (j * 10 + mi * 2 + hh) % 2 == 0:
                        nc.vector.tensor_copy(dst, pT[:, :])
                    else:
                        nc.scalar.copy(dst, pT[:, :])

            # stage 2: W-conv
            p2 = {}
            for name in MAPS:
                for hh in range(2):
                    p = ps2.tile([128, 512], F32, tag="p2")
                    nc.tensor.matmul(p[0:won, :], _band_w(bands_t, j),
                                     t_tiles[name][0:128,
                                                   512 * hh:512 * hh + 512],
                                     start=True, stop=True)
                    p2[(name, hh)] = p

            # combine per half
            for hh in range(2):
                n = won
                si_p, sj_p = p2[("si", hh)], p2[("sj", hh)]
                sij_p, sii_p, sjj_p = (p2[("sij", hh)], p2[("sii", hh)],
                                       p2[("sjj", hh)])
                si_sb = ctmp.tile([128, 512], BF16, tag="si_sb")
                nc.scalar.copy(si_sb[0:n, :], si_p[0:n, :])
                P = ctmp.tile([128, 512], BF16, tag="P")
                nc.vector.tensor_tensor(out=P[0:n, :], in0=si_sb[0:n, :],
                                        in1=sj_p[0:n, :], op=ALU.mult)
                # (P must read p2 psum -> stays on DVE)
                crossN = ctmp.tile([128, 512], BF16, tag="crossN")
                nc.vector.tensor_tensor(out=crossN[0:n, :],
                                        in0=sij_p[0:n, :], in1=P[0:n, :],
                                        op=ALU.subtract)
                si2 = ctmp.tile([128, 512], BF16, tag="si2")
                nc.gpsimd.tensor_tensor(out=si2[0:n, :], in0=si_sb[0:n, :],
                                        in1=si_sb[0:n, :], op=ALU.mult)
                IvarN = ctmp.tile([128, 512], BF16, tag="IvarN")
                nc.vector.tensor_tensor(out=IvarN[0:n, :], in0=sii_p[0:n, :],
                                        in1=si2[0:n, :], op=ALU.subtract)
                sj2 = ctmp.tile([128, 512], BF16, tag="sj2")
                nc.scalar.square(sj2[0:n, :], sj_p[0:n, :])
                JvarN = ctmp.tile([128, 512], BF16, tag="JvarN")
                nc.vector.tensor_tensor(out=JvarN[0:n, :], in0=sjj_p[0:n, :],
                                        in1=sj2[0:n, :], op=ALU.subtract)
                denom = ctmp.tile([128, 512], F32, tag="denom")
                nc.gpsimd.tensor_tensor(out=denom[0:n, :], in0=IvarN[0:n, :],
                                        in1=JvarN[0:n, :], op=ALU.mult)
                rs = ctmp.tile([128, 512], BF16, tag="rs")
                _act_raw(nc, rs[0:n, :], denom[0:n, :], ACTF.Rsqrt)
                q = ctmp.tile([128, 512], BF16, tag="q")
                nc.vector.tensor_tensor(out=q[0:n, :], in0=crossN[0:n, :],
                                        in1=rs[0:n, :], op=ALU.mult)
                qj = ctmp.tile([128, 512], BF16, tag="qj")
                col = ACC_CC + 2 * j + hh
                nc.vector.scalar_tensor_tensor(
                    out=qj[0:n, :], in0=q[0:n, :], scalar=1.0,
                    in1=q[0:n, :], op0=ALU.mult, op1=ALU.mult,
                    accum_out=acc[0:n, col:col + 1])

            emit_s_tile()

        while s_tiles_done[0] < 16:
            emit_s_tile()

        # ---------------- final partition reduction ------------------------
        pF = psF.tile([2, NACC], F32, tag="pF")
        nc.tensor.matmul(pF[:], onesp_t[:], acc[:], start=True, stop=True)
        outt = accp.tile([2, NACC], F32, tag="outt")
        nc.scalar.copy(outt[:], pF[:])
        nc.scalar.dma_start(part_d, outt[:])

    return


def _get_nc():
    if "nc" not in _nc_cache:
        nc = bass.Bass("TRN2", target_bir_lowering=False, debug=False)
        _build(nc)
        _legalize_waits(nc)
        _nc_cache["nc"] = nc
    return _nc_cache["nc"]


def kernel(I, J, s, sum_filt):
    B = I.shape[0]
    assert I.shape == (B, 1, H, W) and s.shape == (B, 2, H, W)
    nc = _get_nc()
    consts = _make_host_consts()

    in_maps = []
    for b in range(B):
        m = {
            "I": np.ascontiguousarray(I[b, 0]),
            "J": np.ascontiguousarray(J[b, 0]),
            "s0": np.ascontiguousarray(s[b, 0]),
            "s1": np.ascontiguousarray(s[b, 1]),
        }
        m.update(_const_map(consts))
        in_maps.append(m)
    res = bass_utils.run_bass_kernel_spmd(nc, in_maps,
                                          core_ids=list(range(B)))
    parts = np.stack([res.results[b]["partials"] for b in range(B)])
    parts = parts.astype(np.float64)  # [B, 2, NACC]

    s64 = s.astype(np.float64)
    cc_sum = float(parts[:, 0, ACC_CC:ACC_CC + 18].sum())
    s2_full = parts[:, 0, ACC_S2:ACC_S2 + 16]        # [B, 16]
    s2_mask = parts[:, 1, ACC_S2:ACC_S2 + 16]
    lag_w = parts[:, 0, ACC_LW:ACC_LW + 16].sum(axis=1)
    A = (parts[:, 0, ACC_SH::2] + parts[:, 0, ACC_SH + 1::2])  # [B, 16]
    lag_h = ((A - s2_full - s2_mask) / 2.0).sum(axis=1)
    s2 = s2_full.sum(axis=1)

    # tile-boundary lag_h pairs (rows 127/128, ...) per core
    rb = np.arange(127, H - 1, 128)
    lag_h = lag_h + (s64[:, :, rb, :] * s64[:, :, rb + 1, :]).sum(axis=(1, 2, 3))

    # edge corrections per core (both channels folded together)
    e_w = (s64[:, :, :, 0] ** 2).sum(axis=(1, 2)) + \
          (s64[:, :, :, -1] ** 2).sum(axis=(1, 2))
    e_h = (s64[:, :, 0, :] ** 2).sum(axis=(1, 2)) + \
          (s64[:, :, -1, :] ** 2).sum(axis=(1, 2))

    sum_dx2 = (2.0 * s2 - e_w - 2.0 * lag_w).sum()
    sum_dy2 = (2.0 * s2 - e_h - 2.0 * lag_h).sum()
    cnt = B * 2 * H * (W - 1)

    ncc_loss = -cc_sum / (B * H * W)
    smooth = 0.5 * (sum_dx2 / cnt + sum_dy2 / cnt) * ALPHA
    total = ncc_loss + smooth
    return np.array([total, ncc_loss, smooth], dtype=np.float32)
